# revision 3
# baseline (speedup 1.0000x reference)
"""JPEG layer (nn_JpegLayer) Trainium2 Bass kernel, 8-core data parallel.

Pipeline per image (per core: 4 images of [3,512,512]):
  P1: 3-accum matmuls fold RGB->YCC color mix + H-DCT (+ vertical 2x-pool for
      chroma) ; route-A, out [h'freq, w]
  T1: PE transposes -> [w, h'freq]
  P2: W-DCT (+ horizontal pool fold for chroma) + DC level-shift correction
      via an extra accumulated rank-structured matmul -> coeffs [w'', h']
  Q : e = d*(1/q); round via +/-2^23*1.5 trick; dec = r*q   (DVE/GPSIMD)
  P3: W-IDCT (+ horizontal 2x upsample fold for chroma) -> [w, h']
  T2: PE transposes -> [h', w]
  P4: H-IDCT (+ vertical upsample fold for chroma) + YCC->RGB fold via
      accumulated matmuls + LEVEL plane via ones-matmul -> psum RGB
  out: DVE tensor_scalar (max 0, min 1) psum->sbuf, DMA out.

All matmul data is float32r (TRN2 reduced-precision fp32 path, 1 cyc/row at
N>=256). Forward-path rounding error ~1e-4 rel; set FP32_FWD=True to run the
forward passes in full fp32 (4 cyc/row) if more accuracy is needed.
"""
import sys
sys.path.insert(0, '/opt/trn_rl_repo')
import numpy as np
import concourse.bacc as bacc
import concourse.bass as bass
import concourse.mybir as mybir
import concourse.tile as tile
from concourse import bass_utils

N_CORES = 8
IMG_PER_CORE = 4
H = W = 512
HT = H // 128            # 4 h-tiles per plane
LEVEL = np.float32(128.0 / 255.0)
LEVEL_F = float(LEVEL)
C_ROUND = 12582912.0   # 1.5*2^23: (x+C)-C == round-half-even(x)
F32 = mybir.dt.float32
F32R = mybir.dt.float32r

RGB2YCC = np.array([[0.299, 0.587, 0.114],
                    [-0.168735892, -0.331264108, 0.5],
                    [0.5, -0.418687589, -0.081312411]], dtype=np.float32)
# YCC2RGB columns: Y col = [1,1,1]; cb col = [0,-0.344136286,1.772]; cr col = [1.402,-0.714136286,0]
CB_C = np.array([0.0, -0.344136286, 1.772], dtype=np.float32)
CR_C = np.array([1.402, -0.714136286, 0.0], dtype=np.float32)


def _dct8():
    i = np.arange(8)[:, None].astype(np.float64)
    j = np.arange(8)[None, :].astype(np.float64)
    m = np.sqrt(2.0 / 8) * np.cos(np.pi * (2 * j + 1) * i / 16.0)
    m[0, :] = 1.0 / np.sqrt(8.0)
    return m.astype(np.float32)


def _blockdiag(b, reps):
    r, c = b.shape
    out = np.zeros((r * reps, c * reps), dtype=np.float32)
    for k in range(reps):
        out[k * r:(k + 1) * r, k * c:(k + 1) * c] = b
    return out


def _build_consts(quantize):
    D = _dct8()
    BD_T = _blockdiag(D.T, 16)             # [128,128] fwd 1D-DCT as lhsT
    BD = _blockdiag(D, 16)                 # [128,128] inverse
    # pooled fwd: PF[16b+2ii+dh, 8b+u] = D[u,ii]/2    [128, 64]
    pf8 = np.zeros((16, 8), dtype=np.float32)
    for ii in range(8):
        for dh in range(2):
            pf8[2 * ii + dh, :] = D[:, ii] * 0.5
    PF = _blockdiag(pf8, 8)                # [128, 64]
    # upsample inverse: PU[8b+v, 16b+2jj+dw] = D[v,jj]   [64, 128]
    pu8 = np.zeros((8, 16), dtype=np.float32)
    for jj in range(8):
        for dw in range(2):
            pu8[:, 2 * jj + dw] = D[jj, :]     # D.T[v,jj] = D[jj,v]? no:
    # careful: idct y[j] = sum_v D[v,j] z[v]  => PU[v, col(j,dw)] = D[v, j]
    pu8 = np.zeros((8, 16), dtype=np.float32)
    for jj in range(8):
        for dw in range(2):
            pu8[:, 2 * jj + dw] = D[:, jj]
    PU = _blockdiag(pu8, 8)                # [64, 128]

    consts = {}
    for c in range(3):
        consts[f"w1y{c}"] = RGB2YCC[0, c] * BD_T
        consts[f"w1c{c}"] = np.concatenate(
            [RGB2YCC[1, c] * PF, RGB2YCC[2, c] * PF], axis=1)  # [128,128]
    consts["w2y"] = BD_T
    consts["w2c"] = PF                     # [128, 64]
    consts["w3y"] = BD
    consts["w3c"] = PU                     # [64, 128]
    consts["w4y"] = BD
    w4 = {}
    for name, cb, cr in (("R", CB_C[0], CR_C[0]), ("G", CB_C[1], CR_C[1]),
                         ("B", CB_C[2], CR_C[2])):
        m = np.zeros((128, 128), dtype=np.float32)
        m[0:64, :] = cb * PU
        m[64:128, :] = cr * PU
        consts[f"w4c{name}"] = m
    consts["ident"] = np.eye(128, dtype=np.float32)

    # quant tables: q = round(quantize[0]*255)/255 (f32, all channels)
    q = (np.round(quantize[0].astype(np.float32) * np.float32(255.0))
         / np.float32(255.0)).astype(np.float32)
    rq = (1.0 / q.astype(np.float64)).astype(np.float32)
    consts["rqt"] = np.tile(rq.T, (16, 64)).astype(np.float32)   # [128,512]
    consts["qt"] = np.tile(q.T, (16, 64)).astype(np.float32)
    # DC correction: coeff d_true = d - 8L*delta00. Via accumulated matmul:
    # lhsT dccor [128,128]: col p (p%8==0) = -8L/128 ; rhs pat8 [128,512]:
    # pat8[k, n] = 1 if n%8==0 else 0  -> psum[p,n] += -8L*d(p%8=0)*d(n%8=0)
    dccor = np.zeros((128, 128), dtype=np.float32)
    dccor[:, 0::8] = np.float32(-8.0 * LEVEL / 128.0)
    consts["dccor"] = dccor
    pat8 = np.zeros((128, 512), dtype=np.float32)
    pat8[:, 0::8] = 1.0
    consts["pat8"] = pat8
    # LEVEL plane: lhsT lones [128,128] all L/128, rhs ones [128,512]
    consts["lones"] = np.full((128, 128), LEVEL / np.float32(128.0),
                              dtype=np.float32)
    consts["ones"] = np.ones((128, 512), dtype=np.float32)
    return consts


_CONST_SHAPES = None


def _build_nc():
    nc = bacc.Bacc("TRN2", target_bir_lowering=False, debug=False,
                   enable_asserts=False, num_devices=N_CORES)
    x_d = nc.dram_tensor("x", [IMG_PER_CORE, 3, H, W], F32R,
                         kind="ExternalInput").ap()
    out_d = nc.dram_tensor("out", [IMG_PER_CORE, 3, H, W], F32,
                           kind="ExternalOutput").ap()
    cd = {}
    for name, shape in _CONST_SHAPES.items():
        cd[name] = nc.dram_tensor(name, list(shape), F32R,
                                  kind="ExternalInput").ap()

    with tile.TileContext(nc) as tc:
        with tc.tile_pool(name="consts", bufs=1) as cp, \
             tc.tile_pool(name="xin", bufs=14) as xp, \
             tc.tile_pool(name="work", bufs=5) as wp, \
             tc.tile_pool(name="stage", bufs=4) as sp, \
             tc.tile_pool(name="psmm", bufs=2, space="PSUM") as pmm, \
             tc.tile_pool(name="pstp", bufs=2, space="PSUM") as ptp:

            cs = {}
            for name, shape in _CONST_SHAPES.items():
                cs[name] = cp.tile(list(shape), F32R, tag=f"c_{name}", name=f"c_{name}")
                nc.sync.dma_start(cs[name][:], cd[name])

            ACT = mybir.ActivationFunctionType
            OP = mybir.AluOpType

            for img in range(IMG_PER_CORE):
                # ---- load RGB tiles ----
                X = {}
                for c in range(3):
                    for t in range(HT):
                        xt = xp.tile([128, 512], F32R, tag="x", name=f"x_{img}_{c}_{t}")
                        nc.sync.dma_start(
                            xt[:], x_d[img, c, 128 * t:128 * (t + 1), :])
                        X[c, t] = xt

                # ---- P1: color + H-DCT (+v-pool chroma) ----
                d1y, d1c = [], []
                for t in range(HT):
                    psY = pmm.tile([128, 512], F32, tag="mm", name="psmm_t")
                    for c in range(3):
                        nc.tensor.matmul(psY[:], cs[f"w1y{c}"][:], X[c, t][:],
                                         start=(c == 0), stop=(c == 2))
                    ty = wp.tile([128, 512], F32R, tag="d1y", name=f"d1y_{img}_{t}")
                    nc.scalar.activation(ty[:], psY[:], ACT.Copy)
                    d1y.append(ty)
                    psC = pmm.tile([128, 512], F32, tag="mm", name="psmm_t")
                    for c in range(3):
                        nc.tensor.matmul(psC[:], cs[f"w1c{c}"][:], X[c, t][:],
                                         start=(c == 0), stop=(c == 2))
                    tcc = wp.tile([128, 512], F32R, tag="d1c", name=f"d1c_{img}_{t}")
                    nc.vector.tensor_copy(tcc[:], psC[:])
                    d1c.append(tcc)

                # ---- T1 ----
                t1y, t1c = [], []
                for s in range(4):
                    pty = ptp.tile([128, 512], F32R, tag="tp", name="pstp_t")
                    for t in range(HT):
                        nc.tensor.transpose(
                            pty[:, 128 * t:128 * (t + 1)],
                            d1y[t][:, 128 * s:128 * (s + 1)], cs["ident"][:])
                    sy = wp.tile([128, 512], F32R, tag="t1y", name=f"t1y_{img}_{s}")
                    nc.scalar.activation(sy[:], pty[:], ACT.Copy)
                    t1y.append(sy)
                    ptc = ptp.tile([128, 512], F32R, tag="tp", name="pstp_t")
                    for t in range(HT):
                        nc.tensor.transpose(
                            ptc[:, 128 * t:128 * (t + 1)],
                            d1c[t][:, 128 * s:128 * (s + 1)], cs["ident"][:])
                    sc = wp.tile([128, 512], F32R, tag="t1c", name=f"t1c_{img}_{s}")
                    nc.vector.tensor_copy(sc[:], ptc[:])
                    t1c.append(sc)

                # ---- P2 + quantize ----
                decy, decc = [], []
                for s in range(4):
                    ps = pmm.tile([128, 512], F32, tag="mm", name="psmm_t")
                    nc.tensor.matmul(ps[:], cs["w2y"][:], t1y[s][:],
                                     start=True, stop=False)
                    nc.tensor.matmul(ps[:], cs["dccor"][:], cs["pat8"][:],
                                     start=False, stop=True)
                    ey = wp.tile([128, 512], F32R, tag="ey", name=f"ey_{img}_{s}")
                    nc.vector.tensor_tensor(ey[:], ps[:], cs["rqt"][:], OP.mult)
                    nc.gpsimd.tensor_scalar(ey[:], ey[:], C_ROUND, C_ROUND,
                                            OP.add, OP.subtract)
                    dy = wp.tile([128, 512], F32R, tag="decy", name=f"decy_{img}_{s}")
                    nc.vector.tensor_tensor(dy[:], ey[:], cs["qt"][:], OP.mult)
                    decy.append(dy)

                    psc = pmm.tile([64, 512], F32, tag="mmc", name="psmmc_t")
                    nc.tensor.matmul(psc[:], cs["w2c"][:], t1c[s][:],
                                     start=True, stop=True)
                    ec = wp.tile([64, 512], F32R, tag="ec", name=f"ec_{img}_{s}")
                    nc.vector.tensor_tensor(ec[:], psc[:], cs["rqt"][0:64, :],
                                            OP.mult)
                    nc.gpsimd.tensor_scalar(ec[:], ec[:], C_ROUND, C_ROUND,
                                            OP.add, OP.subtract)
                    dc = wp.tile([64, 512], F32R, tag="decc", name=f"decc_{img}_{s}")
                    nc.vector.tensor_tensor(dc[:], ec[:], cs["qt"][0:64, :],
                                            OP.mult)
                    decc.append(dc)

                # ---- P3 ----
                p3y, p3c = [], []
                for s in range(4):
                    ps = pmm.tile([128, 512], F32, tag="mm", name="psmm_t")
                    nc.tensor.matmul(ps[:], cs["w3y"][:], decy[s][:],
                                     start=True, stop=True)
                    vy = wp.tile([128, 512], F32R, tag="p3y", name=f"p3y_{img}_{s}")
                    nc.scalar.activation(vy[:], ps[:], ACT.Copy)
                    p3y.append(vy)
                    psc = pmm.tile([128, 512], F32, tag="mm", name="psmm_t")
                    nc.tensor.matmul(psc[:], cs["w3c"][:], decc[s][:],
                                     start=True, stop=True)
                    vc = wp.tile([128, 512], F32R, tag="p3c", name=f"p3c_{img}_{s}")
                    nc.scalar.activation(vc[:], psc[:], ACT.Copy)
                    p3c.append(vc)

                # ---- T2 ----
                t2y, t2c = [], []
                for t in range(4):
                    pty = ptp.tile([128, 512], F32R, tag="tp", name="pstp_t")
                    for s in range(4):
                        nc.tensor.transpose(
                            pty[:, 128 * s:128 * (s + 1)],
                            p3y[s][:, 128 * t:128 * (t + 1)], cs["ident"][:])
                    sy = wp.tile([128, 512], F32R, tag="t2y", name=f"t2y_{img}_{t}")
                    nc.scalar.activation(sy[:], pty[:], ACT.Copy)
                    t2y.append(sy)
                    ptc = ptp.tile([128, 512], F32R, tag="tp", name="pstp_t")
                    for s in range(4):
                        nc.tensor.transpose(
                            ptc[:, 128 * s:128 * (s + 1)],
                            p3c[s][:, 128 * t:128 * (t + 1)], cs["ident"][:])
                    sc = wp.tile([128, 512], F32R, tag="t2c", name=f"t2c_{img}_{t}")
                    nc.vector.tensor_copy(sc[:], ptc[:])
                    t2c.append(sc)

                # ---- P4 + color back + LEVEL + clamp + store ----
                for t in range(4):
                    for ci, cname in enumerate(("R", "G", "B")):
                        ps = pmm.tile([128, 512], F32, tag="mm", name="psmm_t")
                        nc.tensor.matmul(ps[:], cs["w4y"][:], t2y[t][:],
                                         start=True, stop=False)
                        nc.tensor.matmul(ps[:], cs[f"w4c{cname}"][:], t2c[t][:],
                                         start=False, stop=False)
                        nc.tensor.matmul(ps[:], cs["lones"][:], cs["ones"][:],
                                         start=False, stop=True)
                        og = sp.tile([128, 512], F32, tag="og", name=f"og_{img}_{t}_{ci}")
                        nc.vector.tensor_scalar(og[:], ps[:], 0.0, 1.0,
                                                OP.max, OP.min)
                        nc.sync.dma_start(
                            out_d[img, ci, 128 * t:128 * (t + 1), :], og[:])
    nc.compile()
    return nc


_NC_CACHE = None
TRACE = False
LAST_RESULT = None


def kernel(input, quantize):
    global _NC_CACHE, _CONST_SHAPES, LAST_RESULT
    input = np.asarray(input, dtype=np.float32)
    quantize = np.asarray(quantize, dtype=np.float32)
    consts = _build_consts(quantize)
    if _CONST_SHAPES is None:
        _CONST_SHAPES = {k: v.shape for k, v in consts.items()}
    if _NC_CACHE is None:
        _NC_CACHE = _build_nc()
    nc = _NC_CACHE

    in_maps = []
    for core in range(N_CORES):
        shard = np.ascontiguousarray(
            input[core * IMG_PER_CORE:(core + 1) * IMG_PER_CORE])
        m = {"x": shard}
        m.update(consts)
        in_maps.append(m)
    res = bass_utils.run_bass_kernel_spmd(nc, in_maps,
                                          core_ids=list(range(N_CORES)),
                                          trace=TRACE)
    LAST_RESULT = res
    out = np.concatenate([res.results[i]["out"] for i in range(N_CORES)],
                         axis=0)
    return out.astype(np.float32)



# revision 7
# speedup vs baseline: 2.9218x; 2.9218x over previous
"""JPEG layer (nn_JpegLayer) Trainium2 Bass kernel, 8-core data parallel.

Host pre-shifts input by -128/255 (RGB->Y row sums to 1, chroma rows sum
to 0, so this implements the JPEG level shift exactly) and adds it back to
the output; on-device clamp uses shifted bounds. This removes the DC
correction and LEVEL-plane matmuls of the naive formulation.

Per image (per core: 4 images of [3,512,512]):
  poolH: chroma horizontal 2x pool on DVE (strided adds); vertical pool is
         folded into the chroma V-DCT weights -> chroma runs at 1/4 volume.
  P1  : V-DCT, 3-accum matmuls fold the RGB->YCC color mix. Y [128,512]x4,
        C (cb|cr packed in partitions) [128,256]x4.
  T1  : PE transposes -> w into partitions.
  P2  : H-DCT (plain blockdiag, chroma already pooled) -> psum [freq dom].
  Q   : DVE: e = d*(1/q); round via +/-1.5*2^23 trick (one 2-op
        tensor_scalar); dec = r*q stored BF16.
  INV : fused IDCT-H + un-transpose in ONE bf16 matmul per 128-chunk using
        dec as lhsT (out = dec_chunk.T @ BD). Chroma rhs also folds the 2x
        horizontal upsample (N=256). 1 cyc/row at any N in bf16.
  P4  : V-IDCT + YCC->RGB fold via 2-accum matmuls (chroma lhsT folds the
        vertical upsample); DVE clamp to [-L, 1-L]; DMA out.

Matmul data is float32r through the quantize-critical forward path; the
post-quantize inverse runs in bf16 (error ~3e-3 absolute, well within the
2e-2 budget).
"""
import sys
sys.path.insert(0, '/opt/trn_rl_repo')
import numpy as np
import concourse.bacc as bacc
import concourse.bass as bass
import concourse.mybir as mybir
import concourse.tile as tile
from concourse import bass_utils

N_CORES = 8
IMG_PER_CORE = 4
H = W = 512
HT = H // 128            # 4 h-tiles per plane
LEVEL = np.float32(128.0 / 255.0)
C_ROUND = 12582912.0     # 1.5*2^23: (x+C)-C == round-half-even(x)
F32 = mybir.dt.float32
F32R = mybir.dt.float32r
BF16 = mybir.dt.bfloat16

RGB2YCC = np.array([[0.299, 0.587, 0.114],
                    [-0.168735892, -0.331264108, 0.5],
                    [0.5, -0.418687589, -0.081312411]], dtype=np.float32)
# YCC2RGB columns: Y col = [1,1,1]; cb col = [0,-0.344136286,1.772]; cr col = [1.402,-0.714136286,0]
CB_C = np.array([0.0, -0.344136286, 1.772], dtype=np.float32)
CR_C = np.array([1.402, -0.714136286, 0.0], dtype=np.float32)


def _dct8():
    i = np.arange(8)[:, None].astype(np.float64)
    j = np.arange(8)[None, :].astype(np.float64)
    m = np.sqrt(2.0 / 8) * np.cos(np.pi * (2 * j + 1) * i / 16.0)
    m[0, :] = 1.0 / np.sqrt(8.0)
    return m.astype(np.float32)


def _blockdiag(b, reps):
    r, c = b.shape
    out = np.zeros((r * reps, c * reps), dtype=np.float32)
    for k in range(reps):
        out[k * r:(k + 1) * r, k * c:(k + 1) * c] = b
    return out


def _build_consts(quantize):
    D = _dct8()
    BD_T = _blockdiag(D.T, 16)             # [128,128] fwd 1D-DCT as lhsT
    BD = _blockdiag(D, 16)                 # [128,128] inverse
    # chroma fwd: V-pool + V-DCT: pf8[2*ii+dh, u] = D[u,ii]/2   [16,8]
    pf8 = np.zeros((16, 8), dtype=np.float32)
    for ii in range(8):
        for dh in range(2):
            pf8[2 * ii + dh, :] = D[:, ii] * 0.5
    PF = _blockdiag(pf8, 8)                # [128, 64]
    # chroma inv: H/V-IDCT + 2x upsample: pu8[v, 2*jj+dw] = D[v,jj]  [8,16]
    pu8 = np.zeros((8, 16), dtype=np.float32)
    for jj in range(8):
        for dw in range(2):
            pu8[:, 2 * jj + dw] = D[:, jj]
    PU64 = _blockdiag(pu8, 8)              # [64, 128]

    consts = {}
    for c in range(3):
        consts[f"w1y{c}"] = RGB2YCC[0, c] * BD_T
        # extra 0.5: the DVE H-pool is a sum, not a mean
        consts[f"w1c{c}"] = 0.5 * np.concatenate(
            [RGB2YCC[1, c] * PF, RGB2YCC[2, c] * PF], axis=1)  # [128,128]
    consts["bdT"] = BD_T
    consts["ident"] = np.eye(128, dtype=np.float32)
    consts["bd_bf"] = BD                   # loaded as bf16
    consts["pu_bf"] = _blockdiag(pu8, 16)  # [128, 256], loaded as bf16
    consts["w4y"] = BD
    for name, cb, cr in (("R", CB_C[0], CR_C[0]), ("G", CB_C[1], CR_C[1]),
                         ("B", CB_C[2], CR_C[2])):
        m = np.zeros((128, 128), dtype=np.float32)
        m[0:64, :] = cb * PU64
        m[64:128, :] = cr * PU64
        consts[f"w4c{name}"] = m

    # quant tables: q = round(quantize[0]*255)/255 (f32, all channels)
    q = (np.round(quantize[0].astype(np.float32) * np.float32(255.0))
         / np.float32(255.0)).astype(np.float32)
    rq = (1.0 / q.astype(np.float64)).astype(np.float32)
    # freq-domain tiles are [wf partitions, (t|s)*vf cols]: value rq[u=n%8, v=p%8]
    consts["rqt"] = np.tile(rq.T, (16, 64)).astype(np.float32)   # [128,512]
    consts["qt"] = np.tile(q.T, (16, 64)).astype(np.float32)
    return consts


_CONST_SHAPES = None
_CONST_DTYPES = {"bd_bf": BF16, "pu_bf": BF16, "rqt": F32, "qt": F32}


def _build_nc():
    nc = bacc.Bacc("TRN2", target_bir_lowering=False, debug=False,
                   enable_asserts=False, num_devices=N_CORES)
    x_d = nc.dram_tensor("x", [IMG_PER_CORE, 3, H, W], F32R,
                         kind="ExternalInput").ap()
    out_d = nc.dram_tensor("out", [IMG_PER_CORE, 3, H, W], F32,
                           kind="ExternalOutput").ap()
    cd = {}
    for name, shape in _CONST_SHAPES.items():
        cd[name] = nc.dram_tensor(name, list(shape),
                                  _CONST_DTYPES.get(name, F32R),
                                  kind="ExternalInput").ap()

    ACT = mybir.ActivationFunctionType
    OP = mybir.AluOpType

    with tile.TileContext(nc) as tc:
        with tc.tile_pool(name="consts", bufs=1) as cp, \
             tc.tile_pool(name="xin", bufs=14) as xp_pool, \
             tc.tile_pool(name="work", bufs=5) as wp, \
             tc.tile_pool(name="decp", bufs=8) as dp, \
             tc.tile_pool(name="stage", bufs=5) as sp, \
             tc.tile_pool(name="ps1", bufs=1, space="PSUM") as ps1, \
             tc.tile_pool(name="ps2", bufs=2, space="PSUM") as ps2:
            pa = pb = pd = ps1
            pc = ps2

            cs = {}
            for name, shape in _CONST_SHAPES.items():
                cs[name] = cp.tile(list(shape), _CONST_DTYPES.get(name, F32R),
                                   tag=f"c_{name}", name=f"c_{name}")
                nc.sync.dma_start(cs[name][:], cd[name])

            for img in range(IMG_PER_CORE):
                # ---- load RGB tiles ----
                X = {}
                for c in range(3):
                    for t in range(HT):
                        xt = xp_pool.tile([128, 512], F32R, tag="x",
                                          name=f"x_{img}_{c}_{t}")
                        nc.sync.dma_start(
                            xt[:], x_d[img, c, 128 * t:128 * (t + 1), :])
                        X[c, t] = xt

                # ---- chroma H-pool (DVE, strided sum; /4 mean is in w1c) ----
                XP = {}
                for c in range(3):
                    for t in range(HT):
                        xpt = xp_pool.tile([128, 256], F32R, tag="xp",
                                           name=f"xp_{img}_{c}_{t}")
                        nc.vector.tensor_tensor(
                            xpt[:], X[c, t][:, 0::2], X[c, t][:, 1::2],
                            OP.add)
                        XP[c, t] = xpt

                # ---- P1: color mix + V-DCT ----
                d1y, d1c = [], []
                for t in range(HT):
                    psY = pa.tile([128, 512], F32, tag="p1y", name="ps_p1y")
                    for c in range(3):
                        nc.tensor.matmul(psY[:], cs[f"w1y{c}"][:], X[c, t][:],
                                         start=(c == 0), stop=(c == 2))
                    ty = wp.tile([128, 512], F32R, tag="d1y",
                                 name=f"d1y_{img}_{t}")
                    nc.scalar.activation(ty[:], psY[:], ACT.Copy)
                    d1y.append(ty)
                for t in range(HT):
                    psC = pa.tile([128, 256], F32, tag="p1c", name="ps_p1c")
                    for c in range(3):
                        nc.tensor.matmul(psC[:], cs[f"w1c{c}"][:], XP[c, t][:],
                                         start=(c == 0), stop=(c == 2))
                    tcc = wp.tile([128, 256], F32R, tag="d1c",
                                  name=f"d1c_{img}_{t}")
                    nc.scalar.activation(tcc[:], psC[:], ACT.Copy)
                    d1c.append(tcc)

                # ---- T1: w into partitions ----
                t1y, t1c = [], []
                for s in range(4):
                    pty = pb.tile([128, 512], F32R, tag="t1", name="ps_t1")
                    for t in range(HT):
                        nc.tensor.transpose(
                            pty[:, 128 * t:128 * (t + 1)],
                            d1y[t][:, 128 * s:128 * (s + 1)], cs["ident"][:])
                    sy = wp.tile([128, 512], F32R, tag="t1y",
                                 name=f"t1y_{img}_{s}")
                    nc.scalar.activation(sy[:], pty[:], ACT.Copy)
                    t1y.append(sy)
                for s in range(2):
                    ptc = pb.tile([128, 512], F32R, tag="t1", name="ps_t1")
                    for t in range(HT):
                        nc.tensor.transpose(
                            ptc[:, 128 * t:128 * (t + 1)],
                            d1c[t][:, 128 * s:128 * (s + 1)], cs["ident"][:])
                    sc = wp.tile([128, 512], F32R, tag="t1c",
                                 name=f"t1c_{img}_{s}")
                    nc.scalar.activation(sc[:], ptc[:], ACT.Copy)
                    t1c.append(sc)

                # ---- P2: H-DCT + quantize (dec stored bf16) ----
                decy, decc = [], []
                for k in range(6):
                    rhs = t1y[k] if k < 4 else t1c[k - 4]
                    ps = pc.tile([128, 512], F32, tag="q", name="ps_q")
                    nc.tensor.matmul(ps[:], cs["bdT"][:], rhs[:],
                                     start=True, stop=True)
                    e = wp.tile([128, 512], F32, tag="e", name=f"e_{img}_{k}")
                    nc.vector.tensor_tensor(e[:], ps[:], cs["rqt"][:], OP.mult)
                    nc.vector.tensor_scalar(e[:], e[:], C_ROUND, C_ROUND,
                                            OP.add, OP.subtract)
                    dec = dp.tile([128, 512], BF16, tag="dec",
                                  name=f"dec_{img}_{k}")
                    nc.vector.tensor_tensor(dec[:], e[:], cs["qt"][:], OP.mult)
                    (decy if k < 4 else decc).append(dec)

                # ---- INV: fused IDCT-H + un-transpose (bf16, dec as lhsT) ----
                t2y, t2c = [], []
                for t in range(HT):
                    piy = pd.tile([128, 512], F32, tag="inv", name="ps_inv")
                    for s in range(4):
                        nc.tensor.matmul(
                            piy[:, 128 * s:128 * (s + 1)],
                            decy[s][:, 128 * t:128 * (t + 1)], cs["bd_bf"][:],
                            start=True, stop=True)
                    sy = sp.tile([128, 512], F32R, tag="t2y",
                                 name=f"t2y_{img}_{t}")
                    nc.scalar.activation(sy[:], piy[:], ACT.Copy)
                    t2y.append(sy)
                    pic = pd.tile([128, 512], F32, tag="inv", name="ps_inv")
                    for s in range(2):
                        nc.tensor.matmul(
                            pic[:, 256 * s:256 * (s + 1)],
                            decc[s][:, 128 * t:128 * (t + 1)], cs["pu_bf"][:],
                            start=True, stop=True)
                    sc = sp.tile([128, 512], F32R, tag="t2c",
                                 name=f"t2c_{img}_{t}")
                    nc.scalar.activation(sc[:], pic[:], ACT.Copy)
                    t2c.append(sc)

                # ---- P4: V-IDCT + color mix + clamp + store ----
                for t in range(HT):
                    for ci, cname in enumerate(("R", "G", "B")):
                        ps = pc.tile([128, 512], F32, tag="o", name="ps_o")
                        nc.tensor.matmul(ps[:], cs["w4y"][:], t2y[t][:],
                                         start=True, stop=False)
                        nc.tensor.matmul(ps[:], cs[f"w4c{cname}"][:],
                                         t2c[t][:], start=False, stop=True)
                        og = sp.tile([128, 512], F32, tag="og",
                                     name=f"og_{img}_{t}_{ci}")
                        nc.vector.tensor_scalar(og[:], ps[:],
                                                float(-LEVEL),
                                                float(1.0 - LEVEL),
                                                OP.max, OP.min)
                        nc.sync.dma_start(
                            out_d[img, ci, 128 * t:128 * (t + 1), :], og[:])
    nc.compile()
    return nc


_NC_CACHE = None
TRACE = False
LAST_RESULT = None


def kernel(input, quantize):
    global _NC_CACHE, _CONST_SHAPES, LAST_RESULT
    input = np.asarray(input, dtype=np.float32)
    quantize = np.asarray(quantize, dtype=np.float32)
    consts = _build_consts(quantize)
    consts["bd_bf"] = consts["bd_bf"].astype(np.float32)
    if _CONST_SHAPES is None:
        _CONST_SHAPES = {k: v.shape for k, v in consts.items()}
    if _NC_CACHE is None:
        _NC_CACHE = _build_nc()
    nc = _NC_CACHE

    import ml_dtypes
    for name in ("bd_bf", "pu_bf"):
        consts[name] = consts[name].astype(ml_dtypes.bfloat16)

    shifted = input - LEVEL
    in_maps = []
    for core in range(N_CORES):
        shard = np.ascontiguousarray(
            shifted[core * IMG_PER_CORE:(core + 1) * IMG_PER_CORE])
        m = {"x": shard}
        m.update(consts)
        in_maps.append(m)
    res = bass_utils.run_bass_kernel_spmd(nc, in_maps,
                                          core_ids=list(range(N_CORES)),
                                          trace=TRACE)
    LAST_RESULT = res
    out = np.concatenate([res.results[i]["out"] for i in range(N_CORES)],
                         axis=0)
    return (out + LEVEL).astype(np.float32)


# revision 15
# speedup vs baseline: 2.9585x; 1.0126x over previous
"""JPEG layer (nn_JpegLayer) Trainium2 Bass kernel, 8-core data parallel.

Host pre-shifts input by -128/255 (RGB->Y row sums to 1, chroma rows sum
to 0, so this implements the JPEG level shift exactly) and adds it back to
the output; on-device clamp uses shifted bounds. This removes the DC
correction and LEVEL-plane matmuls of the naive formulation.

Per image (per core: 4 images of [3,512,512]):
  poolH: chroma horizontal 2x pool on DVE (strided adds); vertical pool is
         folded into the chroma V-DCT weights -> chroma runs at 1/4 volume.
  P1  : V-DCT, 3-accum matmuls fold the RGB->YCC color mix. Y [128,512]x4,
        C (cb|cr packed in partitions) [128,256]x4.
  T1  : PE transposes -> w into partitions.
  P2  : H-DCT (plain blockdiag, chroma already pooled) -> psum [freq dom].
  Q   : DVE: e = d*(1/q); round via +/-1.5*2^23 trick (one 2-op
        tensor_scalar); dec = r*q stored BF16.
  INV : fused IDCT-H + un-transpose in ONE bf16 matmul per 128-chunk using
        dec as lhsT (out = dec_chunk.T @ BD). Chroma rhs also folds the 2x
        horizontal upsample (N=256). 1 cyc/row at any N in bf16.
  P4  : V-IDCT + YCC->RGB fold via 2-accum matmuls (chroma lhsT folds the
        vertical upsample); DVE clamp to [-L, 1-L]; DMA out.

Matmul data is float32r through the quantize-critical forward path; the
post-quantize inverse runs in bf16 (error ~3e-3 absolute, well within the
2e-2 budget).
"""
import sys
sys.path.insert(0, '/opt/trn_rl_repo')
import numpy as np
import concourse.bacc as bacc
import concourse.bass as bass
import concourse.mybir as mybir
import concourse.tile as tile
from concourse import bass_utils

N_CORES = 8
IMG_PER_CORE = 4
H = W = 512
HT = H // 128            # 4 h-tiles per plane
LEVEL = np.float32(128.0 / 255.0)
C_ROUND = 12582912.0     # 1.5*2^23: (x+C)-C == round-half-even(x)
F32 = mybir.dt.float32
F32R = mybir.dt.float32r
BF16 = mybir.dt.bfloat16

RGB2YCC = np.array([[0.299, 0.587, 0.114],
                    [-0.168735892, -0.331264108, 0.5],
                    [0.5, -0.418687589, -0.081312411]], dtype=np.float32)
# YCC2RGB columns: Y col = [1,1,1]; cb col = [0,-0.344136286,1.772]; cr col = [1.402,-0.714136286,0]
CB_C = np.array([0.0, -0.344136286, 1.772], dtype=np.float32)
CR_C = np.array([1.402, -0.714136286, 0.0], dtype=np.float32)


def _dct8():
    i = np.arange(8)[:, None].astype(np.float64)
    j = np.arange(8)[None, :].astype(np.float64)
    m = np.sqrt(2.0 / 8) * np.cos(np.pi * (2 * j + 1) * i / 16.0)
    m[0, :] = 1.0 / np.sqrt(8.0)
    return m.astype(np.float32)


def _blockdiag(b, reps):
    r, c = b.shape
    out = np.zeros((r * reps, c * reps), dtype=np.float32)
    for k in range(reps):
        out[k * r:(k + 1) * r, k * c:(k + 1) * c] = b
    return out


def _build_consts(quantize):
    D = _dct8()
    BD_T = _blockdiag(D.T, 16)             # [128,128] fwd 1D-DCT as lhsT
    BD = _blockdiag(D, 16)                 # [128,128] inverse
    # chroma fwd: V-pool + V-DCT: pf8[2*ii+dh, u] = D[u,ii]/2   [16,8]
    pf8 = np.zeros((16, 8), dtype=np.float32)
    for ii in range(8):
        for dh in range(2):
            pf8[2 * ii + dh, :] = D[:, ii] * 0.5
    PF = _blockdiag(pf8, 8)                # [128, 64]
    # chroma inv: H/V-IDCT + 2x upsample: pu8[v, 2*jj+dw] = D[v,jj]  [8,16]
    pu8 = np.zeros((8, 16), dtype=np.float32)
    for jj in range(8):
        for dw in range(2):
            pu8[:, 2 * jj + dw] = D[:, jj]
    PU64 = _blockdiag(pu8, 8)              # [64, 128]

    consts = {}
    for c in range(3):
        consts[f"w1y{c}"] = RGB2YCC[0, c] * BD_T
        # extra 0.5: the DVE H-pool is a sum, not a mean
        consts[f"w1c{c}"] = 0.5 * np.concatenate(
            [RGB2YCC[1, c] * PF, RGB2YCC[2, c] * PF], axis=1)  # [128,128]
    consts["bdT"] = BD_T
    consts["ident"] = np.eye(128, dtype=np.float32)
    consts["bd_bf"] = BD                   # loaded as bf16
    consts["pu_bf"] = _blockdiag(pu8, 16)  # [128, 256], loaded as bf16
    consts["w4y"] = BD
    for name, cb, cr in (("R", CB_C[0], CR_C[0]), ("G", CB_C[1], CR_C[1]),
                         ("B", CB_C[2], CR_C[2])):
        m = np.zeros((128, 128), dtype=np.float32)
        m[0:64, :] = cb * PU64
        m[64:128, :] = cr * PU64
        consts[f"w4c{name}"] = m

    # quant tables: q = round(quantize[0]*255)/255 (f32, all channels)
    q = (np.round(quantize[0].astype(np.float32) * np.float32(255.0))
         / np.float32(255.0)).astype(np.float32)
    rq = (1.0 / q.astype(np.float64)).astype(np.float32)
    # freq-domain tiles are [wf partitions, (t|s)*vf cols]: value rq[u=n%8, v=p%8]
    consts["rqt"] = np.tile(rq.T, (16, 64)).astype(np.float32)   # [128,512]
    consts["qt"] = np.tile(q.T, (16, 64)).astype(np.float32)
    return consts


_CONST_SHAPES = None
_CONST_DTYPES = {"bd_bf": BF16, "pu_bf": BF16, "rqt": F32, "qt": F32,
                 "w4y": BF16, "w4cR": BF16, "w4cG": BF16, "w4cB": BF16}


def _build_nc():
    nc = bacc.Bacc("TRN2", target_bir_lowering=False, debug=False,
                   enable_asserts=False, num_devices=N_CORES)
    x_d = nc.dram_tensor("x", [IMG_PER_CORE, 3, H, W], F32R,
                         kind="ExternalInput").ap()
    out_d = nc.dram_tensor("out", [IMG_PER_CORE, 3, H, W], BF16,
                           kind="ExternalOutput").ap()
    cd = {}
    for name, shape in _CONST_SHAPES.items():
        cd[name] = nc.dram_tensor(name, list(shape),
                                  _CONST_DTYPES.get(name, F32R),
                                  kind="ExternalInput").ap()

    ACT = mybir.ActivationFunctionType
    OP = mybir.AluOpType

    with tile.TileContext(nc) as tc:
        with tc.tile_pool(name="consts", bufs=1) as cp, \
             tc.tile_pool(name="xin", bufs=14) as xp_pool, \
             tc.tile_pool(name="work", bufs=5) as wp, \
             tc.tile_pool(name="decp", bufs=8) as dp, \
             tc.tile_pool(name="stage", bufs=5) as sp, \
             tc.tile_pool(name="ps1", bufs=1, space="PSUM") as ps1, \
             tc.tile_pool(name="ps2", bufs=2, space="PSUM") as ps2:
            pa = pb = pd = ps1
            pc = ps2

            cs = {}
            for name, shape in _CONST_SHAPES.items():
                cs[name] = cp.tile(list(shape), _CONST_DTYPES.get(name, F32R),
                                   tag=f"c_{name}", name=f"c_{name}")
                nc.sync.dma_start(cs[name][:], cd[name])

            for img in range(IMG_PER_CORE):
                # ---- load RGB tiles (t-major: P1[t=0] starts after 3 loads) ----
                X = {}
                for t in range(HT):
                    for c in range(3):
                        xt = xp_pool.tile([128, 512], F32R, tag="x",
                                          name=f"x_{img}_{c}_{t}")
                        nc.sync.dma_start(
                            xt[:], x_d[img, c, 128 * t:128 * (t + 1), :])
                        X[c, t] = xt

                # ---- chroma H-pool (DVE, strided sum; /4 mean is in w1c) ----
                XP = {}
                for c in range(3):
                    for t in range(HT):
                        xpt = xp_pool.tile([128, 256], F32R, tag="xp",
                                           name=f"xp_{img}_{c}_{t}")
                        nc.vector.tensor_tensor(
                            xpt[:], X[c, t][:, 0::2], X[c, t][:, 1::2],
                            OP.add)
                        XP[c, t] = xpt

                # ---- P1: color mix + V-DCT ----
                d1y, d1c = [], []
                for t in range(HT):
                    psY = pa.tile([128, 512], F32, tag="p1y", name="ps_p1y")
                    for c in range(3):
                        nc.tensor.matmul(psY[:], cs[f"w1y{c}"][:], X[c, t][:],
                                         start=(c == 0), stop=(c == 2))
                    ty = wp.tile([128, 512], F32R, tag="d1y",
                                 name=f"d1y_{img}_{t}")
                    nc.scalar.activation(ty[:], psY[:], ACT.Copy)
                    d1y.append(ty)
                for t in range(HT):
                    psC = pa.tile([128, 256], F32, tag="p1c", name="ps_p1c")
                    for c in range(3):
                        nc.tensor.matmul(psC[:], cs[f"w1c{c}"][:], XP[c, t][:],
                                         start=(c == 0), stop=(c == 2))
                    tcc = wp.tile([128, 256], F32R, tag="d1c",
                                  name=f"d1c_{img}_{t}")
                    nc.scalar.activation(tcc[:], psC[:], ACT.Copy)
                    d1c.append(tcc)

                # ---- T1: w into partitions ----
                t1y, t1c = [], []
                for s in range(4):
                    pty = pb.tile([128, 512], F32R, tag="t1", name="ps_t1")
                    for t in range(HT):
                        nc.tensor.transpose(
                            pty[:, 128 * t:128 * (t + 1)],
                            d1y[t][:, 128 * s:128 * (s + 1)], cs["ident"][:])
                    sy = wp.tile([128, 512], F32R, tag="t1y",
                                 name=f"t1y_{img}_{s}")
                    nc.scalar.activation(sy[:], pty[:], ACT.Copy)
                    t1y.append(sy)
                for s in range(2):
                    ptc = pb.tile([128, 512], F32R, tag="t1", name="ps_t1")
                    for t in range(HT):
                        nc.tensor.transpose(
                            ptc[:, 128 * t:128 * (t + 1)],
                            d1c[t][:, 128 * s:128 * (s + 1)], cs["ident"][:])
                    sc = wp.tile([128, 512], F32R, tag="t1c",
                                 name=f"t1c_{img}_{s}")
                    nc.scalar.activation(sc[:], ptc[:], ACT.Copy)
                    t1c.append(sc)

                # ---- P2: H-DCT + quantize (dec stored bf16) ----
                decy, decc = [], []
                for k in range(6):
                    rhs = t1y[k] if k < 4 else t1c[k - 4]
                    ps = pc.tile([128, 512], F32, tag="q", name="ps_q")
                    nc.tensor.matmul(ps[:], cs["bdT"][:], rhs[:],
                                     start=True, stop=True)
                    e = wp.tile([128, 512], F32, tag="e", name=f"e_{img}_{k}")
                    nc.vector.tensor_tensor(e[:], ps[:], cs["rqt"][:], OP.mult)
                    nc.vector.tensor_scalar(e[:], e[:], C_ROUND, C_ROUND,
                                            OP.add, OP.subtract)
                    dec = dp.tile([128, 512], BF16, tag="dec",
                                  name=f"dec_{img}_{k}")
                    nc.vector.tensor_tensor(dec[:], e[:], cs["qt"][:], OP.mult)
                    (decy if k < 4 else decc).append(dec)

                # ---- INV: fused IDCT-H + un-transpose (bf16, dec as lhsT) ----
                t2y, t2c = [], []
                for t in range(HT):
                    piy = pd.tile([128, 512], F32, tag="inv", name="ps_inv")
                    for s in range(4):
                        nc.tensor.matmul(
                            piy[:, 128 * s:128 * (s + 1)],
                            decy[s][:, 128 * t:128 * (t + 1)], cs["bd_bf"][:],
                            start=True, stop=True)
                    sy = sp.tile([128, 512], BF16, tag="t2y",
                                 name=f"t2y_{img}_{t}")
                    nc.scalar.activation(sy[:], piy[:], ACT.Copy)
                    t2y.append(sy)
                    pic = pd.tile([128, 512], F32, tag="inv", name="ps_inv")
                    for s in range(2):
                        nc.tensor.matmul(
                            pic[:, 256 * s:256 * (s + 1)],
                            decc[s][:, 128 * t:128 * (t + 1)], cs["pu_bf"][:],
                            start=True, stop=True)
                    sc = sp.tile([128, 512], BF16, tag="t2c",
                                 name=f"t2c_{img}_{t}")
                    nc.scalar.activation(sc[:], pic[:], ACT.Copy)
                    t2c.append(sc)

                # ---- P4: V-IDCT + color mix + clamp + store ----
                for t in range(HT):
                    for ci, cname in enumerate(("R", "G", "B")):
                        ps = pc.tile([128, 512], F32, tag="o", name="ps_o")
                        nc.tensor.matmul(ps[:], cs["w4y"][:], t2y[t][:],
                                         start=True, stop=False)
                        nc.tensor.matmul(ps[:], cs[f"w4c{cname}"][:],
                                         t2c[t][:], start=False, stop=True)
                        og = sp.tile([128, 512], BF16, tag="og",
                                     name=f"og_{img}_{t}_{ci}")
                        nc.vector.tensor_scalar(og[:], ps[:],
                                                float(-LEVEL),
                                                float(1.0 - LEVEL),
                                                OP.max, OP.min)
                        nc.sync.dma_start(
                            out_d[img, ci, 128 * t:128 * (t + 1), :], og[:])
    nc.compile()
    return nc


_NC_CACHE = None
TRACE = False
LAST_RESULT = None


def kernel(input, quantize):
    global _NC_CACHE, _CONST_SHAPES, LAST_RESULT
    input = np.asarray(input, dtype=np.float32)
    quantize = np.asarray(quantize, dtype=np.float32)
    consts = _build_consts(quantize)
    consts["bd_bf"] = consts["bd_bf"].astype(np.float32)
    if _CONST_SHAPES is None:
        _CONST_SHAPES = {k: v.shape for k, v in consts.items()}
    if _NC_CACHE is None:
        _NC_CACHE = _build_nc()
    nc = _NC_CACHE

    import ml_dtypes
    for name in ("bd_bf", "pu_bf", "w4y", "w4cR", "w4cG", "w4cB"):
        consts[name] = consts[name].astype(ml_dtypes.bfloat16)

    shifted = input - LEVEL
    in_maps = []
    for core in range(N_CORES):
        shard = np.ascontiguousarray(
            shifted[core * IMG_PER_CORE:(core + 1) * IMG_PER_CORE])
        m = {"x": shard}
        m.update(consts)
        in_maps.append(m)
    res = bass_utils.run_bass_kernel_spmd(nc, in_maps,
                                          core_ids=list(range(N_CORES)),
                                          trace=TRACE)
    LAST_RESULT = res
    out = np.concatenate([res.results[i]["out"].astype(np.float32)
                          for i in range(N_CORES)], axis=0)
    return (out + LEVEL).astype(np.float32)
